# revision 25
# baseline (speedup 1.0000x reference)
"""Trainium2 Bass kernel: AttentionAggregator (GNN message passing).

Reference math per branch (user / item):
    cat  = concat_k [ tabA[adjA[:,k]] | tabB[adjB[:,k]] ]      # [NJ, 256]
    S    = (q @ q.T) / sqrt(D) + 1                             # [NJ, NJ]
    agg  = softmax(S, axis=-1) @ cat                           # [NJ, NJ] @ [NJ, 256]
    out  = relu(agg @ W)                                       # [NJ, 64]

Refactorings:
  * (softmax(S) @ cat) @ W == softmax(S) @ (cat @ W).  VW = cat @ W is a
    cheap [NJ, 256]@[256, 64] GEMM computed on HOST (like the baseline's
    host-projected TW tables, one step further: the gather and 8-way sum
    also happen on host).  The device sees VW as a plain DMA-able
    parameter -- no on-device gather, no AllGather, no collectives.
  * softmax is shift-invariant: a GLOBAL constant shift of the scores
    multiplies numerator and denominator rows by the same factor, so
    row = exp(S - SHIFT) @ [VW | 1] normalizes to the same output.
    SHIFT centers exp(S) into fp8e5m2 range.
  * The diagonal softmax weight (j == m) carries up to ~30% of a row's
    mass, so its fp8 error does not average out like the other 8191
    terms; the host epilogue replaces the (emulated) quantized diagonal
    term with the exact one, then normalizes: relu(num/den).T.

Sharding: 8 cores, row-parallel. Cores 0-3 take 2048-row slices of the user
branch, cores 4-7 of the item branch; one SPMD program, different data.

Per-core dataflow (engines in parentheses):
  S^T tiles:  s[j-tile 128, m 512] = qT_t.T @ qmT -- K=32 contraction, so
              four j-tiles' matmuls are packed in the PE array
              concurrently via row tiling (tile_position=(32c, 0));
              qT/qmT are partition-replicated x4 so each row-group has
              its operands at its own base partition.  (PE)
  exp:        E = exp(S/sqrt(D)+1-SHIFT) in fp8e5m2.  PSUM ring
              [128, 7, 512] (7 banks) used as a fixed double buffer:
              N=2048 ACTIVATEs over slots 0-3, N=1024 over slots 4-5 --
              while ACT reads one side the PE fills the other, so every
              ACTIVATE's inputs are ready a full instruction early and
              ACT runs back-to-back (it is the bottleneck: ~134us).
  out GEMM:   out_ps[80, 512] += [vw_t | vw_t+1].T @ [E_t | E_t+1] --
              fp8 DoubleRow matmuls contract TWO j-tiles (K=2x128) per
              512-col stream, halving PE streaming so the PE keeps pace
              with ACT even at the cold 1.2 GHz HAM clock.  vw rows are
              fp8e4m3, padded to 80 cols (16B LDW stride rule).  (PE)
  store:      out_ps (1 bank) -> SBUF -> DRAM [65, M] fp32 per m-block.
"""

import os
import sys

sys.path.insert(0, "/opt/trn_rl_repo")
os.environ.setdefault("MYCRO_LOCAL_CACHE", "1")

import numpy as np

import concourse.bass as bass
import concourse.bacc as bacc
import concourse.mybir as mybir
import concourse.tile as tile

try:  # ml_dtypes ships with jax
    import ml_dtypes

    BF16_NP = ml_dtypes.bfloat16
    FP8E4_NP = ml_dtypes.float8_e4m3
except ImportError:  # pragma: no cover
    BF16_NP = None
    FP8E4_NP = None

P = 128
HB = 512   # matmul free width (one PSUM bank of fp32) == m-block width
O1P = 80   # padded vw row: 64 proj + 1 ones + 15 zero (16B stride rule)
SHIFT = 6.5  # global score shift (cancels in softmax normalization)

# Schraudolph fast-exp constants for the DVE path (small spans):
#   exp(s*inv_sqrt_d + 1 - SHIFT) ~= bitcast_f32(int32(K1*s + K2))
# The ~2% sawtooth error averages out over each softmax row's ~8k terms
# (its mean component cancels in num/den); diagonal terms are repaired by
# the host correction, which emulates this exact bit path.
SCH_C = 370000.0  # minimax-ish bias tune
DVE_TRUNC = True  # device f32->i32 convert semantics (flip if HW rounds)
_LOG2E = float(np.log2(np.e))


def _sch_consts(D):
    k1 = (2.0 ** 23) * _LOG2E / float(np.sqrt(D))
    k2 = (2.0 ** 23) * (127.0 + _LOG2E * (1.0 - SHIFT)) - SCH_C
    return k1, k2


def _spans_for(JT):
    """[4, 2] ACT spans per m-block: big spans fill ring slots 0-3 (one
    N=2048 ACTIVATE), small spans slots 4-5 (N=1024).  Even span sizes
    mean every DoubleRow G pair (t, t+1) is adjacent -- no permutation.
    Returns [(t0, n_tiles, slot0), ...]."""
    spans = []
    t = 0
    while t < JT:
        n = 4 if JT - t >= 4 else JT - t
        slot0 = 0 if n == 4 else 4
        spans.append((t, n, slot0))
        t += n
        if t < JT and JT - t >= 2:
            spans.append((t, 2, 4))
            t += 2
    assert all(n % 2 == 0 for _, n, _ in spans)
    return spans


class Cfg:
    def __init__(self, NJ=8192, M=2048, D=32, K=4, OUT=64,
                 RW=2, E4B=10, E2B=10, NDUM=10, DVE_EXP=True):
        self.DVE_EXP = DVE_EXP  # small spans' exp on DVE (Schraudolph)
        self.NJ = NJ      # attention length (rows of the branch)
        self.M = M        # rows this core owns
        self.D = D        # embedding dim (contraction for scores)
        self.K = K        # neighbors per adjacency list
        self.OUT = OUT    # output dim
        self.RW = RW      # G lag in PERIODS (span pairs)
        self.E4B = E4B    # [P, 4, HB] exp-tile pool bufs
        self.E2B = E2B    # [P, 2, HB] exp-tile pool bufs
        self.NDUM = NDUM  # dummy matmuls to warm the PE clock ramp
        self.JT = NJ // P             # j-tiles
        self.NMB = M // HB            # m-blocks (512 wide)
        assert NJ % P == 0 and M % HB == 0
        assert D * 4 == P


def build_nc(cfg: Cfg) -> bass.Bass:
    NJ, M, D, OUT = cfg.NJ, cfg.M, cfg.D, cfg.OUT
    JT, NMB = cfg.JT, cfg.NMB
    RW, E4B, E2B, NDUM = cfg.RW, cfg.E4B, cfg.E2B, cfg.NDUM
    O1 = OUT + 1
    bf16 = mybir.dt.bfloat16
    fp32 = mybir.dt.float32
    fp8e4 = mybir.dt.float8e4
    fp8e5 = mybir.dt.float8e5

    spans = _spans_for(JT)
    nc = bacc.Bacc(num_devices=8, num_swdge_queues=4)

    # partition-replicated x4 (rows 32c+d = qT[d]) so row-tiled matmuls
    # find lhsT/rhs at base partitions 0/32/64/96
    qT = nc.declare_dram_parameter("qT", [P, NJ], bf16, isOutput=False)
    qmT = nc.declare_dram_parameter("qmT", [P, M], bf16, isOutput=False)
    # host-computed neighbor projection:
    # vw[p, t, 0:64] = (cat @ W)[t*128 + p], [.., 64] = 1, rest 0
    vw = nc.declare_dram_parameter("vw", [P, JT, O1P], fp8e4, isOutput=False)
    # un-normalized output: rows 0..63 = exp(S') @ VW, row 64 = exp(S') @ 1
    out = nc.declare_dram_parameter("out", [O1, M], fp32, isOutput=True)

    inv_sqrt_d = 1.0 / float(np.sqrt(D))
    SCH_K1, SCH_K2 = _sch_consts(D)
    QCH = 1024  # qT DMA chunk width

    with tile.TileContext(nc) as tc:
        with (
            tc.tile_pool(name="const", bufs=1) as const_pool,
            tc.tile_pool(name="e4", bufs=E4B) as e4_pool,
            tc.tile_pool(name="e2", bufs=E2B) as e2_pool,
            tc.tile_pool(name="i32", bufs=3) as i32_pool,
            tc.tile_pool(name="osb", bufs=2) as osb_pool,
            tc.tile_pool(name="ring", bufs=1, space="PSUM") as ring_pool,
            tc.tile_pool(name="ops", bufs=1, space="PSUM") as ops_pool,
        ):
            # ---- constants / persistent SBUF tensors -----------------------
            qT_sb = const_pool.tile([P, NJ], bf16, tag="qT_sb")
            qmT_sb = const_pool.tile([P, M], bf16, tag="qmT_sb")
            vw_sb = const_pool.tile([P, JT, O1P], fp8e4, tag="vw_sb")
            # load order = consumption order: first m-block rhs, first
            # j-tiles, the vw table, then the rest of qT/qmT
            nc.sync.dma_start(out=qmT_sb[:, 0:HB], in_=qmT[:, 0:HB])
            PRE = min(2 * QCH, NJ)  # prefetched before the vw table
            nc.sync.dma_start(out=qT_sb[:, 0:HB], in_=qT[:, 0:HB])
            for c0 in range(HB, PRE, HB):
                nc.sync.dma_start(out=qT_sb[:, c0:c0 + HB],
                                  in_=qT[:, c0:c0 + HB])
            nc.sync.dma_start(out=vw_sb[:], in_=vw[:, :, :])
            for c0 in range(PRE, NJ, QCH):
                nc.sync.dma_start(out=qT_sb[:, c0:c0 + QCH],
                                  in_=qT[:, c0:c0 + QCH])
            if M > HB:
                nc.sync.dma_start(out=qmT_sb[:, HB:M], in_=qmT[:, HB:M])

            bias_sh = const_pool.tile([P, 1], fp32, tag="bias_sh")
            nc.gpsimd.memset(bias_sh[:], 1.0 - SHIFT)

            # Warm-up Exp so the ACT table-set pseudo-load lands on an
            # instruction with few sync waits, not on the first pipelined
            # exp of the main loop.
            warm = const_pool.tile([P, 1], fp32, tag="warm")
            nc.scalar.activation(
                out=warm[:], in_=bias_sh[:],
                func=mybir.ActivationFunctionType.Exp,
                bias=bias_sh[:, 0:1], scale=1.0)

            # ---- warm-kick: dummy accumulation chain gated on the first
            # DMA chunk latches the HAM clock to 2.4 GHz early (the PE
            # p-state only upgrades on a ~fully-busy 3.4us window, which
            # the dependency-coupled steady state never produces).  The
            # garbage output is overwritten by the first real G matmul
            # (start=True) reusing the same ops-pool bank.
            if NDUM:
                dum_ps = ops_pool.tile([O1P, HB], fp32, tag="out_ps")
                for i in range(NDUM):
                    nc.tensor.matmul(
                        out=dum_ps[:],
                        lhsT=qmT_sb[:, 0:O1P],
                        rhs=qmT_sb[:, 0:HB],
                        start=(i == 0),
                        stop=(i == NDUM - 1),
                        skip_group_check=True,
                    )

            # PSUM: S double buffer as TWO tiles -- the tile framework's
            # WAR tracking is tile-granular, so one 7-slot ring tile would
            # serialize S matmuls against the ACTIVATE reading the other
            # slots.  Separate tiles let the PE fill one side while ACT
            # drains the other.  (4 + 2 banks; + out accum 1 bank.)
            s_big = ring_pool.tile([P, 4, HB], fp32, tag="s_big")
            s_small = ring_pool.tile([P, 2, HB], fp32, tag="s_small")

            # ---- emission helpers -----------------------------------------
            e_store = {}  # (mb, span_idx) -> e tile

            def emit_sx(mb, si):
                t0, n, slot0 = spans[si]
                s_ps = s_big if n == 4 else s_small
                for i in range(n):
                    c = (t0 + i) % 4
                    nc.tensor.matmul(
                        out=s_ps[:, i, :],
                        lhsT=qT_sb[32 * c:32 * (c + 1),
                                   (t0 + i) * P:(t0 + i + 1) * P],
                        rhs=qmT_sb[32 * c:32 * (c + 1),
                                   mb * HB:(mb + 1) * HB],
                        start=True,
                        stop=True,
                        tile_position=(32 * c, 0),
                    )
                pool = e4_pool if n == 4 else e2_pool
                e_sb = pool.tile([P, n, HB], fp8e5, tag=f"e{n}")
                if cfg.DVE_EXP and n == 2:
                    # Schraudolph fast exp on the (otherwise idle) DVE:
                    # i32 = K1*s + K2 (f32 ALU, converted on write), then
                    # reinterpret the int as f32 and cast to fp8e5.
                    i32_sb = i32_pool.tile([P, n, HB], mybir.dt.int32,
                                           tag="i32")
                    nc.vector.tensor_scalar(
                        out=i32_sb[:],
                        in0=s_ps[:, 0:n, :],
                        scalar1=float(SCH_K1),
                        scalar2=float(SCH_K2),
                        op0=mybir.AluOpType.mult,
                        op1=mybir.AluOpType.add,
                    )
                    nc.vector.tensor_copy(
                        out=e_sb[:], in_=i32_sb[:].bitcast(fp32))
                else:
                    nc.scalar.activation(
                        out=e_sb[:],
                        in_=s_ps[:, 0:n, :],
                        func=mybir.ActivationFunctionType.Exp,
                        bias=bias_sh[:, 0:1],
                        scale=inv_sqrt_d,
                    )
                e_store[(mb, si)] = e_sb

            out_pss = {}

            def emit_g(mb, si):
                t0, n, _ = spans[si]
                if mb not in out_pss:
                    out_psn = ops_pool.tile([O1P, HB], fp32, tag="out_ps")
                    out_pss[mb] = out_psn
                e_sb = e_store.pop((mb, si))
                for i in range(0, n, 2):
                    nc.tensor.matmul(
                        out=out_pss[mb][:],
                        lhsT=vw_sb[:, t0 + i:t0 + i + 2, :],
                        rhs=e_sb[:, i:i + 2, :],
                        start=(t0 + i == 0),
                        stop=(t0 + i + 2 == JT),
                        perf_mode=mybir.MatmulPerfMode.DoubleRow,
                        skip_group_check=True,
                    )
                if t0 + n == JT:  # m-block finished: stage + store
                    o_sb = osb_pool.tile([O1, HB], fp32, tag="o_sb")
                    nc.vector.tensor_copy(out=o_sb[:], in_=out_pss[mb][0:O1, :])
                    nc.sync.dma_start(
                        out=out[:, mb * HB:(mb + 1) * HB], in_=o_sb[:])
                    del out_pss[mb]

            # ---- runway then steady lag-RW pipeline -----------------------
            # Periods pair a big span with the following small span.  Both
            # spans' S matmuls are emitted BEFORE the lagged G matmuls so
            # the in-order PE produces the ACTIVATE inputs first and fills
            # the remaining time with G work.
            periods = []
            for si, (_, n, _) in enumerate(spans):
                if n == 4:
                    periods.append([si])
                else:
                    periods[-1].append(si) if periods else periods.append([si])
            NP = len(periods)
            sched = [(mb, p) for mb in range(NMB) for p in range(NP)]
            for i, (mb, p) in enumerate(sched):
                for si in periods[p]:
                    emit_sx(mb, si)
                if i >= RW:
                    gmb, gp = sched[i - RW]
                    for si in periods[gp]:
                        emit_g(gmb, si)
            for i in range(max(0, len(sched) - RW), len(sched)):
                gmb, gp = sched[i]
                for si in periods[gp]:
                    emit_g(gmb, si)

    nc.finalize()
    return nc


# --------------------------------------------------------------------------
# host side
# --------------------------------------------------------------------------

def _bf16(a: np.ndarray) -> np.ndarray:
    return np.ascontiguousarray(a.astype(BF16_NP))


def _make_vw(tabA, tabB, adjA, adjB, W, cfg: Cfg):
    """vw[p, t, c] param: host gather + projection, ones col, zero pad.
    Also returns the exact fp32 projection for the diagonal correction."""
    f32 = np.float32
    NJ, K, OUT = cfg.NJ, cfg.K, cfg.OUT
    # cat = concat_k [tabA[adjA[:,k]] | tabB[adjB[:,k]]]  -> [NJ, K*2D]
    parts = []
    for k in range(K):
        parts.append(tabA[adjA[:, k]])
        parts.append(tabB[adjB[:, k]])
    cat = np.concatenate(parts, axis=1).astype(f32)          # [NJ, 2KD]
    vw_full = cat @ W.astype(f32)                            # [NJ, OUT]
    vw1 = np.zeros((NJ, O1P), f32)
    vw1[:, :OUT] = vw_full
    vw1[:, OUT] = 1.0
    vw1 = vw1.reshape(cfg.JT, P, O1P).transpose(1, 0, 2)     # [P, JT, O1P]
    return np.ascontiguousarray(vw1.astype(FP8E4_NP)), vw_full


def _sch_exp_dev(s, D):
    """Bit-exact emulation of the device DVE Schraudolph path."""
    f32 = np.float32
    k1, k2 = _sch_consts(D)
    t = (s.astype(f32) * f32(k1)).astype(f32) + f32(k2)
    i = np.trunc(t) if DVE_TRUNC else np.round(t)
    return i.astype(np.int64).astype(np.int32).view(f32)


def _diag_correction(qvecs, vw_full, cfg: Cfg):
    """The diagonal softmax weight carries up to ~30% of a row's mass, so
    its fp8 quantization error does not average out.  The host knows q_m
    and VW row m exactly: subtract the device's (emulated) quantized
    diagonal term and add back the exact one.
    Returns (num_adj [64, NJ], den_adj [NJ])."""
    f32 = np.float32
    qb = np.asarray(qvecs).astype(BF16_NP).astype(f32)       # device q
    s_psum = (qb * qb).sum(1)                                 # device S diag
    s_diag = s_psum / np.sqrt(f32(cfg.D)) + 1.0 - SHIFT
    w_ex = np.exp(s_diag, dtype=f32)
    w_f32 = np.exp(s_diag, dtype=f32)
    if cfg.DVE_EXP:
        # rows whose j-tile falls in a small span took the DVE fast-exp
        spans = _spans_for(cfg.JT)
        on_dve = np.zeros(cfg.NJ, bool)
        for t0, n, _ in spans:
            if n == 2:
                on_dve[t0 * P:(t0 + n) * P] = True
        w_f32 = np.where(on_dve, _sch_exp_dev(s_psum, cfg.D), w_f32)
    w_dev = w_f32.astype(ml_dtypes.float8_e5m2).astype(f32)  # device E8 diag
    v_ex = vw_full                                            # [NJ, 64]
    v_dev = vw_full.astype(FP8E4_NP).astype(f32)
    num_adj = w_ex * v_ex.T - w_dev * v_dev.T                # [64, NJ]
    den_adj = w_ex - w_dev
    return num_adj, den_adj


def _make_in_maps(cfg: Cfg, review_vecs, user_vecs, item_vecs,
                  adj_ur, adj_ri, adj_ir, adj_ru,
                  user_neigh_W, item_neigh_W, n_cores=8):
    half = n_cores // 2
    D = cfg.D
    # cat column order per j: [ur_0 | ri_0 | ur_1 | ri_1 | ...] (K blocks
    # of 2D) -- concat([ur, ri], axis=2).reshape in the reference.
    uT = np.tile(_bf16(np.asarray(user_vecs).T), (P // D, 1))
    iT = np.tile(_bf16(np.asarray(item_vecs).T), (P // D, 1))
    vw_u, vwf_u = _make_vw(np.asarray(review_vecs), np.asarray(item_vecs),
                           np.asarray(adj_ur), np.asarray(adj_ri),
                           np.asarray(user_neigh_W), cfg)
    vw_i, vwf_i = _make_vw(np.asarray(review_vecs), np.asarray(user_vecs),
                           np.asarray(adj_ir), np.asarray(adj_ru),
                           np.asarray(item_neigh_W), cfg)
    corr_u = _diag_correction(user_vecs, vwf_u, cfg)
    corr_i = _diag_correction(item_vecs, vwf_i, cfg)

    in_maps, corrs = [], []
    for core in range(n_cores):
        if core < half:  # user branch
            qTb, vwb, corr = uT, vw_u, corr_u
        else:  # item branch
            qTb, vwb, corr = iT, vw_i, corr_i
        s = (core % half) * cfg.M
        in_maps.append({
            "qT": qTb,
            "qmT": np.ascontiguousarray(qTb[:, s:s + cfg.M]),
            "vw": vwb,
        })
        corrs.append((corr[0][:, s:s + cfg.M], corr[1][s:s + cfg.M]))
    return in_maps, corrs


_BUILT = {}


def _get_nc(cfg: Cfg) -> bass.Bass:
    key = tuple(sorted(cfg.__dict__.items()))
    if key not in _BUILT:
        _BUILT[key] = build_nc(cfg)
    return _BUILT[key]


def kernel(review_vecs, user_vecs, item_vecs, adj_ur, adj_ri, adj_ir, adj_ru,
           user_neigh_W, item_neigh_W, _trace=False):
    from concourse.bass_utils import run_bass_kernel_spmd

    n_cores = 8
    Nu = np.asarray(user_vecs).shape[0]
    cfg = Cfg(NJ=Nu, M=Nu // (n_cores // 2))
    nc = _get_nc(cfg)
    in_maps, corrs = _make_in_maps(cfg, review_vecs, user_vecs, item_vecs,
                                   adj_ur, adj_ri, adj_ir, adj_ru,
                                   user_neigh_W, item_neigh_W, n_cores)
    res = run_bass_kernel_spmd(nc, in_maps, core_ids=list(range(n_cores)),
                               trace=_trace)
    # host epilogue: out rows 0..63 = exp(S') @ VW, row 64 = denominator;
    # exact-diagonal correction, then normalize + relu
    branch = []
    for i in range(n_cores):
        o = np.asarray(res.results[i]["out"], dtype=np.float32)  # [65, M]
        num_adj, den_adj = corrs[i]
        num = o[:64] + num_adj
        den = o[64:65] + den_adj[None, :]
        branch.append(np.maximum(num / den, 0.0).T)              # [M, 64]
    user_out = np.concatenate(branch[: n_cores // 2], axis=0)
    item_out = np.concatenate(branch[n_cores // 2:], axis=0)
    if _trace:
        return (user_out, item_out), res
    return user_out, item_out
